# revision 10
# baseline (speedup 1.0000x reference)
"""Cox partial-likelihood NLL loss on 8 Trainium2 NeuronCores.

Math: with time sorted ascending and c = cumsum(exp(risk)),
    end(i)  = last index of i's tie group
    loss    = -(A - B) / N
    A       = sum_i event[i] * risk[i]
    B       = sum_i event[i] * ln(c[end(i)])

c[end(i)] = min over group-end positions k >= i of c[k] (c is increasing).
Device computes, per core (contiguous chunk, partition-major layout):
  s = exp(risk); cs = partition-local forward add-scan of s
  mb = cs + 1e30 * [time[i] == time[i+1]]     (finite only at group ends)
  bf = reverse min-scan of mb per tile, then hierarchical suffix-min fixup
       (tile suffix -> partition suffix -> cross-core via tiny AllGather)
  B  = sum event * ln(bf + rowbase + corebase)   (STT product + accum)
  A  = sum event * risk                          (PE diag-block matmuls)
Host sums the 8 per-core (A_c, B_c) partials.
"""

import numpy as np
import ml_dtypes

N_FULL = 16_777_216
NCORES_FULL = 8
P = 128

BIG = 1.0e30      # mask offset for non-boundary positions
BIGF = 3.0e38     # "+inf" for f32 min chains


def build_nc(n_cores: int, K: int, F: int):
    """Build the Bass module for per-core chunk length K, tile free-size F."""
    import concourse.bacc as bacc
    import concourse.tile as tile
    import concourse.mybir as mybir

    f32 = mybir.dt.float32
    bf16 = mybir.dt.bfloat16
    i16 = mybir.dt.int16
    Alu = mybir.AluOpType
    Act = mybir.ActivationFunctionType
    X = mybir.AxisListType.X

    FT = K // P          # elements per partition
    assert FT * P == K
    # ramp-up schedule: small leading tiles so compute starts early
    tiles = []
    off = 0
    ramp = [512, 512, 1024, 2048]
    for w in ramp:
        if off + w <= FT and FT >= 4 * F:
            tiles.append((off, w))
            off += w
    while off < FT:
        w = min(F, FT - off)
        tiles.append((off, w))
        off += w
    T = len(tiles)

    nc = bacc.Bacc(
        "TRN2",
        target_bir_lowering=False,
        debug=False,
        enable_asserts=False,
        num_devices=n_cores,
    )

    risk_d = nc.dram_tensor("risk", [K], bf16, kind="ExternalInput").ap()
    event_d = nc.dram_tensor("event", [K], bf16, kind="ExternalInput").ap()
    t16_d = nc.dram_tensor("t16", [K], i16, kind="ExternalInput").ap()
    tn16_d = nc.dram_tensor("tn16", [K], i16, kind="ExternalInput").ap()
    # constants / per-core masks
    m1_d = nc.dram_tensor("m1", [P, P], f32, kind="ExternalInput").ap()
    eye_d = nc.dram_tensor("eye", [P, P], f32, kind="ExternalInput").ap()
    ones1_d = nc.dram_tensor("ones1", [1, P], f32, kind="ExternalInput").ap()
    masklt_d = nc.dram_tensor("masklt", [1, n_cores], f32, kind="ExternalInput").ap()
    pen_d = nc.dram_tensor("pen", [1, n_cores], f32, kind="ExternalInput").ap()
    out_d = nc.dram_tensor("out", [1, 64], f32, kind="ExternalOutput").ap()

    risk2 = risk_d.rearrange("(p f) -> p f", p=P)
    event2 = event_d.rearrange("(p f) -> p f", p=P)
    t162 = t16_d.rearrange("(p f) -> p f", p=P)
    tn162 = tn16_d.rearrange("(p f) -> p f", p=P)

    with tile.TileContext(nc) as tc:
        with (
            tc.tile_pool(name="pers", bufs=1) as pers,
            tc.tile_pool(name="io", bufs=2) as io,
            tc.tile_pool(name="sp", bufs=1) as sp,
            tc.tile_pool(name="pp", bufs=1, space="PSUM") as pp,
            tc.tile_pool(name="dram", bufs=1, space="DRAM") as dram,
        ):
            # ---- persistent SBUF ----
            bf0 = pers.tile([P, FT], bf16)         # mb -> bf (in place)
            event_sb = pers.tile([P, FT], bf16)
            TM = pers.tile([P, T], f32)            # per-tile row mins
            RS = pers.tile([P, T], f32)            # suffix mins over tiles
            ciloc = pers.tile([P, T], f32)         # per-(partition,tile) init
            Bacc = pers.tile([P, T], f32)          # per-tile B partial sums
            m1 = pers.tile([P, P], f32)
            eye = pers.tile([P, P], f32)
            ones1 = pers.tile([1, P], f32)
            masklt = pers.tile([1, n_cores], f32)
            pen = pers.tile([1, n_cores], f32)
            rowbase = pers.tile([P, 1], f32)       # excl prefix of partition totals
            bias128 = pers.tile([P, 1], f32)       # rowbase + base_c
            initloc = pers.tile([P, 1], f32)
            g128 = pers.tile([P, 1], f32)
            gT = pers.tile([1, P], f32)
            rsT = pers.tile([1, P], f32)
            exT = pers.tile([1, P], f32)
            initg = pers.tile([P, 1], f32)
            Pt = pers.tile([1, P], f32)            # partition totals, transposed
            S8 = pers.tile([1, n_cores], f32)
            M8 = pers.tile([1, n_cores], f32)
            bv8 = pers.tile([1, n_cores], f32)     # exclusive prefix of S8
            mg8 = pers.tile([1, n_cores], f32)
            stage = pers.tile([1, 64], f32)        # collective-in / output staging
            scal = pers.tile([1, 8], f32)          # small scalar scratch (p0)
            dAT = pers.tile([1, P], f32)
            dBT = pers.tile([1, P], f32)
            tmpd = pers.tile([P, P], f32)
            dA = pers.tile([P, 1], f32)
            dB = pers.tile([P, 1], f32)

            # ---- PSUM ----
            psumA = pp.tile([P, P], f32)
            psumP = pp.tile([P, 1], f32)

            # ---- DRAM bounce for the collective ----
            cc_in = dram.tile([1, 64], f32)
            cc_out = dram.tile([n_cores, 64], f32)

            # load constants (small)
            nc.sync.dma_start(m1[:], m1_d[:])
            nc.sync.dma_start(eye[:], eye_d[:])
            nc.sync.dma_start(ones1[:], ones1_d[:])
            nc.sync.dma_start(masklt[:], masklt_d[:])
            nc.sync.dma_start(pen[:], pen_d[:])

            # ================= phase 1: streaming =================
            cs_prev = None
            w_prev = None
            for t, (off, w) in enumerate(tiles):
                sl = slice(off, off + w)
                rbf_t = io.tile([P, w], bf16, tag="rbf")
                t16_t = io.tile([P, w], i16, tag="t16")
                tn16_t = io.tile([P, w], i16, tag="tn16")
                eq_t = io.tile([P, w], bf16, tag="eq")
                s_t = sp.tile([P, w], f32, tag="s")
                cs_t = io.tile([P, w], f32, tag="cs")

                nc.sync.dma_start(rbf_t[:], risk2[:, sl])
                nc.sync.dma_start(t16_t[:], t162[:, sl])
                nc.sync.dma_start(tn16_t[:], tn162[:, sl])
                nc.sync.dma_start(event_sb[:, sl], event2[:, sl])

                # s = exp(risk)
                nc.scalar.activation(s_t[:], rbf_t[:], Act.Exp)
                # cs = forward add-scan of s (chained across tiles)
                init = 0.0 if cs_prev is None else cs_prev[:, w_prev - 1 : w_prev]
                nc.vector.tensor_tensor_scan(
                    cs_t[:], s_t[:], s_t[:], init, Alu.add, Alu.bypass
                )
                # eq = (t16 == tn16)  {1.0 interior, 0.0 at group end}
                nc.vector.tensor_tensor(eq_t[:], t16_t[:], tn16_t[:], Alu.is_equal)
                # mb = eq*BIG + cs   (bf16)
                nc.vector.scalar_tensor_tensor(
                    bf0[:, sl], eq_t[:], BIG, cs_t[:], Alu.mult, Alu.add
                )
                # bf0 = reverse min-scan of mb within the tile (in place)
                rev = bf0[:, sl][:, ::-1]
                nc.vector.tensor_tensor_scan(
                    rev, rev, rev, BIGF, Alu.min, Alu.bypass
                )
                # tile row-min = leftmost element of the reverse scan
                nc.vector.tensor_copy(TM[:, t : t + 1], bf0[:, off : off + 1])

                # A += event_blk . risk_blk (diagonal blocks, accumulate)
                for b in range(w // P):
                    bsl = slice(off + b * P, off + (b + 1) * P)
                    nc.tensor.matmul(
                        psumA[:],
                        event_sb[:, bsl],
                        rbf_t[:, b * P : (b + 1) * P],
                        start=(t == 0 and b == 0),
                        stop=(t == T - 1 and b == w // P - 1),
                        skip_group_check=True,
                    )
                cs_prev = cs_t
                w_prev = w

            # ================= mid phase: tiny cross ops =================
            # partition totals P[p] = cs last column; rowbase = strict-lower
            # prefix via PE
            Ptot = cs_prev[:, w_prev - 1 : w_prev]
            nc.tensor.matmul(psumP[:], m1[:], Ptot, start=True, stop=True,
                             skip_group_check=True)
            nc.scalar.copy(rowbase[:], psumP[:])
            # S_c = sum of partition totals (transpose via DMA, reduce on p0)
            nc.sync.dma_start(Pt[:], Ptot)
            nc.vector.tensor_reduce(scal[:, 0:1], Pt[:], X, Alu.add)
            # suffix mins over tiles within each partition
            nc.vector.tensor_tensor_scan(
                RS[:, ::-1], TM[:, ::-1], TM[:, ::-1], BIGF, Alu.min, Alu.bypass
            )
            # whole-core row mins in core-local frame: g = RS[:,0] + rowbase
            nc.vector.tensor_tensor(g128[:], RS[:, 0:1], rowbase[:], Alu.add)
            nc.sync.dma_start(gT[:], g128[:])
            nc.vector.tensor_reduce(scal[:, 1:2], gT[:], X, Alu.min)
            # stage [S_c, M_c'] and AllGather
            nc.vector.memset(stage[:], 0.0)
            nc.vector.tensor_copy(stage[:, 0:2], scal[:, 0:2])
            nc.sync.dma_start(cc_in[:], stage[:])
            nc.gpsimd.collective_compute(
                "AllGather",
                Alu.bypass,
                replica_groups=[list(range(n_cores))],
                ins=[cc_in[:].opt()],
                outs=[cc_out[:].opt()],
            )
            nc.sync.dma_start(S8[:], cc_out[:, 0:1])
            nc.sync.dma_start(M8[:], cc_out[:, 1:2])
            # bv8 = exclusive prefix-sum of S8
            nc.vector.memset(bv8[:], 0.0)
            if n_cores > 1:
                nc.vector.tensor_copy(bv8[:, 1:n_cores], S8[:, 0 : n_cores - 1])
            nc.vector.tensor_tensor_scan(
                bv8[:], bv8[:], bv8[:], 0.0, Alu.add, Alu.bypass
            )
            # base_c = sum(S8 * masklt)
            nc.vector.tensor_tensor(mg8[:], S8[:], masklt[:], Alu.mult)
            nc.vector.tensor_reduce(scal[:, 2:3], mg8[:], X, Alu.add)
            # R_c = min over later cores of (M + base), else BIGF
            nc.vector.tensor_tensor(mg8[:], M8[:], bv8[:], Alu.add)
            nc.vector.tensor_tensor(mg8[:], mg8[:], pen[:], Alu.add)
            nc.vector.tensor_reduce(scal[:, 3:4], mg8[:], X, Alu.min)
            # bias128 = rowbase + base_c (broadcast via PE ones)
            nc.tensor.matmul(psumP[:], ones1[:], scal[:, 2:3], start=True,
                             stop=True, skip_group_check=True)
            nc.vector.tensor_tensor(bias128[:], rowbase[:], psumP[:], Alu.add)
            # R_c in the core-local frame: Rl = R_c - base_c
            nc.vector.tensor_tensor(
                scal[:, 4:5], scal[:, 3:4], scal[:, 2:3], Alu.subtract
            )
            # partition-suffix mins with floor Rl (exclusive)
            nc.vector.tensor_tensor_scan(
                rsT[:, ::-1], gT[:, ::-1], gT[:, ::-1], scal[:, 4:5],
                Alu.min, Alu.bypass,
            )
            nc.vector.tensor_copy(exT[:, 0 : P - 1], rsT[:, 1:P])
            nc.vector.tensor_copy(exT[:, P - 1 : P], scal[:, 4:5])
            nc.sync.dma_start(initg[:], exT[:])
            nc.vector.tensor_tensor(initloc[:], initg[:], rowbase[:], Alu.subtract)
            # ciloc[:, t] = min(RS[:, t+1], initloc)
            nc.vector.memset(ciloc[:], BIGF)
            if T > 1:
                nc.vector.tensor_copy(ciloc[:, 0 : T - 1], RS[:, 1:T])
            nc.vector.tensor_scalar(
                ciloc[:], ciloc[:], initloc[:], None, Alu.min
            )

            # ================= phase 2: fix up + Ln + B accum ===========
            for t, (off, w) in enumerate(tiles):
                sl = slice(off, off + w)
                lbf_t = io.tile([P, w], bf16, tag="lbf")
                nc.vector.tensor_scalar(
                    bf0[:, sl], bf0[:, sl], ciloc[:, t : t + 1], None, Alu.min
                )
                nc.scalar.activation(
                    lbf_t[:], bf0[:, sl], Act.Ln, bias=bias128[:, 0:1], scale=1.0
                )
                nc.vector.scalar_tensor_tensor(
                    lbf_t[:], lbf_t[:], 0.0, event_sb[:, sl],
                    Alu.bypass, Alu.mult,
                    accum_out=Bacc[:, t : t + 1],
                )

            # ================= epilogue: reduce A and B =================
            nc.vector.tensor_tensor(tmpd[:], psumA[:], eye[:], Alu.mult)
            nc.vector.tensor_reduce(dA[:], tmpd[:], X, Alu.add)
            nc.vector.tensor_reduce(dB[:], Bacc[:], X, Alu.add)
            nc.sync.dma_start(dAT[:], dA[:])
            nc.sync.dma_start(dBT[:], dB[:])
            nc.vector.memset(stage[:], 0.0)
            nc.vector.tensor_reduce(stage[:, 0:1], dAT[:], X, Alu.add)
            nc.vector.tensor_reduce(stage[:, 1:2], dBT[:], X, Alu.add)
            nc.vector.tensor_copy(stage[:, 2:6], scal[:, 0:4])
            nc.sync.dma_start(out_d[:], stage[:])

    nc.compile()
    return nc


def _host_prep(risk, event_indicator, time, n_cores, K):
    """Shard + dtype-convert inputs; returns per-core in_maps."""
    tnext = np.empty_like(time)
    tnext[:-1] = time[1:]
    tnext[-1] = time[-1] + 1
    t16 = time.astype(np.int16)
    tn16 = tnext.astype(np.int16)
    # fix any int16 aliasing so (t16==tn16) <=> (time==tnext)
    bad = (tnext != time) & (tn16 == t16)
    if bad.any():
        tn16[bad] = (t16[bad] + 1).astype(np.int16)
    ev16 = event_indicator.astype(ml_dtypes.bfloat16)
    rk16 = risk.astype(ml_dtypes.bfloat16)

    m1 = np.triu(np.ones((P, P), np.float32), 1)  # m1[q, m] = 1 if q < m
    eye = np.eye(P, dtype=np.float32)
    ones1 = np.ones((1, P), np.float32)

    in_maps = []
    for c in range(n_cores):
        sl = slice(c * K, (c + 1) * K)
        masklt = (np.arange(n_cores) < c).astype(np.float32).reshape(1, -1)
        pen = np.where(np.arange(n_cores) > c, 0.0, BIGF).astype(
            np.float32).reshape(1, -1)
        in_maps.append({
            "risk": np.ascontiguousarray(rk16[sl]),
            "event": np.ascontiguousarray(ev16[sl]),
            "t16": np.ascontiguousarray(t16[sl]),
            "tn16": np.ascontiguousarray(tn16[sl]),
            "m1": m1, "eye": eye, "ones1": ones1,
            "masklt": masklt, "pen": pen,
        })
    return in_maps


_NC_CACHE = {}


def _get_nc(n_cores, K, F):
    key = (n_cores, K, F)
    if key not in _NC_CACHE:
        _NC_CACHE[key] = build_nc(n_cores, K, F)
    return _NC_CACHE[key]


def run(risk, event_indicator, time, n_cores=NCORES_FULL, F=4096, **spmd_kwargs):
    from concourse.bass_utils import run_bass_kernel_spmd

    n = risk.shape[0]
    K = n // n_cores
    nc = _get_nc(n_cores, K, F)
    in_maps = _host_prep(risk, event_indicator, time, n_cores, K)
    res = run_bass_kernel_spmd(
        nc, in_maps, core_ids=list(range(n_cores)), **spmd_kwargs
    )
    outs = np.stack([r["out"][0] for r in res.results])  # [n_cores, 64]
    A = outs[:, 0].astype(np.float64).sum()
    B = outs[:, 1].astype(np.float64).sum()
    loss = -(A - B) / n
    return np.float32(loss), res


def kernel(risk, event_indicator, time):
    loss, _ = run(risk, event_indicator, time)
    return np.asarray(loss, dtype=np.float32)


# revision 18
# speedup vs baseline: 1.2699x; 1.2699x over previous
"""Cox partial-likelihood NLL loss on 8 Trainium2 NeuronCores.

Math: with time sorted ascending and c = cumsum(exp(risk)),
    end(i)  = last index of i's tie group
    loss    = -(A - B) / N
    A       = sum_i event[i] * risk[i]
    B       = sum_i event[i] * ln(c[end(i)])

c[end(i)] = min over group-end positions k >= i of c[k] (c is increasing).
Device computes, per core (contiguous chunk, partition-major layout):
  s = exp(risk) (accum -> S_c, AllGathered early, overlapped with compute)
  cs = partition-local forward add-scan of s
  mb = cs + 1e30 * [time[i] == time[i+1]]     (finite only at group ends)
  bf = reverse min-scan of mb per tile, then hierarchical suffix-min fixup
       (tile suffix -> partition suffix; cross-core handled by a HALO tile:
        the next core's first H elements are re-processed locally, so the
        fill value for this core's tail is found without exchanging mins)
  B  = sum event * ln(bf + rowbase + corebase)   (STT product + accum)
  A  = sum event * risk                          (PE diag-block matmuls)
Host sums the 8 per-core (A_c, B_c) partials.
"""

import numpy as np
import ml_dtypes

N_FULL = 16_777_216
NCORES_FULL = 8
P = 128

BIG = 1.0e30      # mask offset for non-boundary positions
BIGF = 3.0e38     # "+inf" for f32 min chains
HW_HALO = 128     # halo tile free-width (halo = 128*HW_HALO elements)


def build_nc(n_cores: int, K: int, F: int):
    """Build the Bass module for per-core chunk length K, tile free-size F."""
    import concourse.bacc as bacc
    import concourse.tile as tile
    import concourse.mybir as mybir

    f32 = mybir.dt.float32
    bf16 = mybir.dt.bfloat16
    i16 = mybir.dt.int16
    Alu = mybir.AluOpType
    Act = mybir.ActivationFunctionType
    X = mybir.AxisListType.X

    FT = K // P          # elements per partition
    assert FT * P == K
    # ramp-up schedule: small leading tiles so compute starts early
    tiles = []
    off = 0
    ramp = [512, 512, 1024, 2048]
    for w in ramp:
        if off + w <= FT and FT >= 4 * F:
            tiles.append((off, w))
            off += w
    while off < FT:
        w = min(F, FT - off)
        tiles.append((off, w))
        off += w
    TM_ = len(tiles)         # number of MAIN tiles
    T = TM_ + 1              # + halo tile
    HW = HW_HALO if FT >= 4 * F else 32
    HK = P * HW              # halo element count

    nc = bacc.Bacc(
        "TRN2",
        target_bir_lowering=False,
        debug=False,
        enable_asserts=False,
        num_devices=n_cores,
    )

    risk_d = nc.dram_tensor("risk", [K], bf16, kind="ExternalInput").ap()
    event_d = nc.dram_tensor("event", [K], bf16, kind="ExternalInput").ap()
    t16_d = nc.dram_tensor("t16", [K], i16, kind="ExternalInput").ap()
    tn16_d = nc.dram_tensor("tn16", [K], i16, kind="ExternalInput").ap()
    hrisk_d = nc.dram_tensor("hrisk", [HK], bf16, kind="ExternalInput").ap()
    ht16_d = nc.dram_tensor("ht16", [HK], i16, kind="ExternalInput").ap()
    htn16_d = nc.dram_tensor("htn16", [HK], i16, kind="ExternalInput").ap()
    # constants / per-core masks
    m1_d = nc.dram_tensor("m1", [P, P], f32, kind="ExternalInput").ap()
    eye_d = nc.dram_tensor("eye", [P, P], f32, kind="ExternalInput").ap()
    ones1_d = nc.dram_tensor("ones1", [1, P], f32, kind="ExternalInput").ap()
    masklt_d = nc.dram_tensor("masklt", [1, n_cores], f32, kind="ExternalInput").ap()
    out_d = nc.dram_tensor("out", [1, 64], f32, kind="ExternalOutput").ap()

    risk2 = risk_d.rearrange("(p f) -> p f", p=P)
    event2 = event_d.rearrange("(p f) -> p f", p=P)
    t162 = t16_d.rearrange("(p f) -> p f", p=P)
    tn162 = tn16_d.rearrange("(p f) -> p f", p=P)
    hrisk2 = hrisk_d.rearrange("(p f) -> p f", p=P)
    ht162 = ht16_d.rearrange("(p f) -> p f", p=P)
    htn162 = htn16_d.rearrange("(p f) -> p f", p=P)

    with tile.TileContext(nc) as tc:
        with (
            tc.tile_pool(name="pers", bufs=1) as pers,
            tc.tile_pool(name="io", bufs=2) as io,
            tc.tile_pool(name="sp", bufs=1) as sp,
            tc.tile_pool(name="pp", bufs=1, space="PSUM") as pp,
            tc.tile_pool(name="dram", bufs=1, space="DRAM") as dram,
        ):
            # ---- persistent SBUF ----
            bf0 = pers.tile([P, FT], bf16)         # mb -> bf (in place)
            event_sb = pers.tile([P, FT], bf16)
            TM = pers.tile([P, TM_], f32)          # per-tile row mins (main)
            RS = pers.tile([P, TM_], f32)          # suffix mins over tiles
            ciloc = pers.tile([P, TM_], f32)       # per-(partition,tile) init
            Bacc = pers.tile([P, TM_], f32)        # per-tile B partial sums
            Eacc = pers.tile([P, TM_], f32)        # per-tile exp row sums
            m1 = pers.tile([P, P], f32)
            eye = pers.tile([P, P], f32)
            ones1 = pers.tile([1, P], f32)
            masklt = pers.tile([1, n_cores], f32)
            rowbase = pers.tile([P, 1], f32)       # excl prefix of partition totals
            bias128 = pers.tile([P, 1], f32)       # rowbase + base_c
            initloc = pers.tile([P, 1], f32)
            g128 = pers.tile([P, 1], f32)
            exT = pers.tile([1, P], f32)
            erow = pers.tile([P, 1], f32)          # per-partition exp sums
            hacc = pers.tile([P, 1], f32)          # halo per-row exp sums
            hrb = pers.tile([P, 1], f32)           # halo row bases
            hmb = pers.tile([P, HW], bf16)         # halo masked values
            hcs = pers.tile([P, HW], f32)
            hmin = pers.tile([P, 1], f32)
            S8 = pers.tile([1, n_cores], f32)
            bv8 = pers.tile([1, n_cores], f32)     # exclusive prefix of S8
            mg8 = pers.tile([1, n_cores], f32)
            stage = pers.tile([1, 64], f32)        # collective-in / output staging
            scal = pers.tile([1, 8], f32)          # small scalar scratch (p0)
            tmpd = pers.tile([P, P], f32)
            dA = pers.tile([P, 1], f32)
            dB = pers.tile([P, 1], f32)

            # ---- PSUM ----
            psumA = pp.tile([P, P], f32)
            psumP = pp.tile([P, 1], f32)
            psumT = pp.tile([1, P], f32)
            psumI = pp.tile([P, 1], f32)

            # ---- DRAM bounce for the collective ----
            cc_in = dram.tile([1, 64], f32)
            cc_out = dram.tile([n_cores, 64], f32)

            nc.vector.memset(scal[:], 0.0)
            nc.vector.memset(Bacc[:], 0.0)
            nc.vector.memset(Eacc[:], 0.0)
            # load constants (small)
            nc.sync.dma_start(m1[:], m1_d[:])
            nc.sync.dma_start(eye[:], eye_d[:])
            nc.sync.dma_start(ones1[:], ones1_d[:])
            nc.sync.dma_start(masklt[:], masklt_d[:])

            # ================= phase 1: streaming =================
            cs_prev = None
            w_prev = None

            for t, (off, w) in enumerate(tiles):
                sl = slice(off, off + w)
                rbf_t = io.tile([P, w], bf16, tag="rbf")
                t16_t = io.tile([P, w], i16, tag="t16")
                tn16_t = io.tile([P, w], i16, tag="tn16")
                eq_t = io.tile([P, w], bf16, tag="eq")
                s_t = sp.tile([P, w], f32, tag="s")
                cs_t = io.tile([P, w], f32, tag="cs")

                nc.sync.dma_start(rbf_t[:], risk2[:, sl])
                nc.sync.dma_start(t16_t[:], t162[:, sl])
                nc.sync.dma_start(tn16_t[:], tn162[:, sl])
                nc.sync.dma_start(event_sb[:, sl], event2[:, sl])

                # s = exp(risk); row sums accumulate toward S_c
                nc.scalar.activation(
                    s_t[:], rbf_t[:], Act.Exp, accum_out=Eacc[:, t : t + 1]
                )
                # cs = forward add-scan of s (chained across tiles)
                init = 0.0 if cs_prev is None else cs_prev[:, w_prev - 1 : w_prev]
                nc.vector.tensor_tensor_scan(
                    cs_t[:], s_t[:], s_t[:], init, Alu.add, Alu.bypass
                )
                # eq = (t16 == tn16)  {1.0 interior, 0.0 at group end}
                nc.vector.tensor_tensor(eq_t[:], t16_t[:], tn16_t[:], Alu.is_equal)
                # mb = eq*BIG + cs   (bf16)
                nc.vector.scalar_tensor_tensor(
                    bf0[:, sl], eq_t[:], BIG, cs_t[:], Alu.mult, Alu.add
                )
                # bf0 = reverse min-scan of mb within the tile (in place)
                rev = bf0[:, sl][:, ::-1]
                nc.vector.tensor_tensor_scan(
                    rev, rev, rev, BIGF, Alu.min, Alu.bypass
                )
                # tile row-min = leftmost element of the reverse scan
                nc.vector.tensor_copy(TM[:, t : t + 1], bf0[:, off : off + 1])

                # A += event_blk . risk_blk (diagonal blocks, accumulate)
                for b in range(w // P):
                    bsl = slice(off + b * P, off + (b + 1) * P)
                    nc.tensor.matmul(
                        psumA[:],
                        event_sb[:, bsl],
                        rbf_t[:, b * P : (b + 1) * P],
                        start=(t == 0 and b == 0),
                        stop=(t == TM_ - 1 and b == w // P - 1),
                        skip_group_check=True,
                    )
                cs_prev = cs_t
                w_prev = w

            # ---- early collective: AllGather core sums S_c (overlapped) ----
            nc.vector.tensor_reduce(erow[:], Eacc[:], X, Alu.add)
            nc.tensor.transpose(psumT[:], erow[:], eye[:])
            nc.vector.tensor_reduce(scal[:, 0:1], psumT[:], X, Alu.add)
            nc.vector.memset(stage[:], 0.0)
            nc.vector.tensor_copy(stage[:, 0:1], scal[:, 0:1])
            nc.sync.dma_start(cc_in[:], stage[:])
            nc.gpsimd.collective_compute(
                "AllGather",
                Alu.bypass,
                replica_groups=[list(range(n_cores))],
                ins=[cc_in[:].opt()],
                outs=[cc_out[:].opt()],
            )
            nc.sync.dma_start(S8[:], cc_out[:, 0:1])
            # bv8 = exclusive prefix-sum of S8; base_c = sum(S8 * masklt)
            nc.vector.memset(bv8[:], 0.0)
            if n_cores > 1:
                nc.vector.tensor_copy(bv8[:, 1:n_cores], S8[:, 0 : n_cores - 1])
            nc.vector.tensor_tensor_scan(
                bv8[:], bv8[:], bv8[:], 0.0, Alu.add, Alu.bypass
            )
            nc.vector.tensor_tensor(mg8[:], S8[:], masklt[:], Alu.mult)
            nc.vector.tensor_reduce(scal[:, 2:3], mg8[:], X, Alu.add)

            # ---- halo chunk (next core's first HK elements) ----
            # Scan it in the true core-global frame: row q's initial is
            # S_local + sum of halo rows < q. Its masked min M_halo is the
            # fill floor for this core's tail (replaces a cross-core min
            # exchange).
            hrbf = io.tile([P, HW], bf16, tag="rbf")
            ht16 = io.tile([P, HW], i16, tag="t16")
            htn16 = io.tile([P, HW], i16, tag="tn16")
            heq = io.tile([P, HW], bf16, tag="eq")
            nc.sync.dma_start(hrbf[:], hrisk2[:, :])
            nc.sync.dma_start(ht16[:], ht162[:, :])
            nc.sync.dma_start(htn16[:], htn162[:, :])
            nc.scalar.activation(hcs[:], hrbf[:], Act.Exp, accum_out=hacc[:])
            # halo row bases: strict-lower prefix of hacc + S_local broadcast
            nc.tensor.matmul(psumI[:], m1[:], hacc[:], start=True, stop=False,
                             skip_group_check=True)
            nc.tensor.matmul(psumI[:], ones1[:], scal[:, 0:1], start=False,
                             stop=True, skip_group_check=True)
            nc.scalar.copy(hrb[:], psumI[:])
            nc.vector.tensor_tensor_scan(
                hcs[:], hcs[:], hcs[:], hrb[:, 0:1], Alu.add, Alu.bypass
            )
            nc.vector.tensor_tensor(heq[:], ht16[:], htn16[:], Alu.is_equal)
            nc.vector.scalar_tensor_tensor(
                hmb[:], heq[:], BIG, hcs[:], Alu.mult, Alu.add
            )
            nc.vector.tensor_reduce(hmin[:], hmb[:], X, Alu.min)
            nc.tensor.transpose(psumT[:], hmin[:], eye[:])
            nc.vector.tensor_reduce(scal[:, 5:6], psumT[:], X, Alu.min)

            # ================= mid phase: local-only cross ops ==========
            # rowbase = excl prefix over partitions of MAIN row totals (erow;
            # ACT-accumulated, ~= scan totals to within fp rounding).
            nc.tensor.matmul(psumP[:], m1[:], erow[:], start=True, stop=True,
                             skip_group_check=True)
            nc.scalar.copy(rowbase[:], psumP[:])
            # suffix mins over tiles within each partition
            nc.vector.tensor_tensor_scan(
                RS[:, ::-1], TM[:, ::-1], TM[:, ::-1], BIGF, Alu.min, Alu.bypass
            )
            # whole-core row mins in core-local frame: g = RS[:,0] + rowbase
            nc.vector.tensor_tensor(g128[:], RS[:, 0:1], rowbase[:], Alu.add)
            nc.tensor.transpose(psumT[:], g128[:], eye[:])
            # partition-suffix mins, exclusive, floor M_halo:
            # exT[p] = min(min over q>p of gT[q], M_halo)
            nc.vector.tensor_tensor_scan(
                exT[:, 0 : P - 1][:, ::-1],
                psumT[:, 1:P][:, ::-1],
                eye[0:1, 0 : P - 1],
                scal[:, 5:6], Alu.min, Alu.bypass,
            )
            nc.vector.tensor_copy(exT[:, P - 1 : P], scal[:, 5:6])
            nc.tensor.transpose(psumI[:], exT[:], eye[0:1, 0:1])
            nc.vector.tensor_tensor(initloc[:], psumI[:], rowbase[:], Alu.subtract)
            # bias128 = rowbase + base_c (broadcast via PE ones)
            nc.tensor.matmul(psumP[:], ones1[:], scal[:, 2:3], start=True,
                             stop=True, skip_group_check=True)
            nc.vector.tensor_tensor(bias128[:], rowbase[:], psumP[:], Alu.add)
            # ciloc[:, t] = min(RS[:, t+1], initloc); last tile: initloc only
            nc.vector.memset(ciloc[:], BIGF)
            if TM_ > 1:
                nc.vector.tensor_copy(ciloc[:, 0 : TM_ - 1], RS[:, 1:TM_])
            nc.vector.tensor_scalar(
                ciloc[:], ciloc[:], initloc[:], None, Alu.min
            )

            # ================= phase 2: fix up + Ln + B accum ===========
            for t, (off, w) in enumerate(tiles):
                sl = slice(off, off + w)
                lbf_t = io.tile([P, w], bf16, tag="lbf")
                nc.vector.tensor_scalar(
                    bf0[:, sl], bf0[:, sl], ciloc[:, t : t + 1], None, Alu.min
                )
                nc.scalar.activation(
                    lbf_t[:], bf0[:, sl], Act.Ln, bias=bias128[:, 0:1], scale=1.0
                )
                nc.vector.scalar_tensor_tensor(
                    lbf_t[:], lbf_t[:], 0.0, event_sb[:, sl],
                    Alu.bypass, Alu.mult,
                    accum_out=Bacc[:, t : t + 1],
                )

            # ================= epilogue: reduce A and B =================
            nc.vector.tensor_tensor(tmpd[:], psumA[:], eye[:], Alu.mult)
            nc.vector.tensor_reduce(dA[:], tmpd[:], X, Alu.add)
            nc.vector.tensor_reduce(dB[:], Bacc[:], X, Alu.add)
            nc.vector.memset(stage[:], 0.0)
            nc.tensor.transpose(psumT[:], dA[:], eye[:])
            nc.vector.tensor_reduce(stage[:, 0:1], psumT[:], X, Alu.add)
            nc.tensor.transpose(psumT[:], dB[:], eye[:])
            nc.vector.tensor_reduce(stage[:, 1:2], psumT[:], X, Alu.add)
            nc.vector.tensor_copy(stage[:, 2:4], scal[:, 0:2])
            nc.vector.tensor_copy(stage[:, 4:5], scal[:, 2:3])
            nc.sync.dma_start(out_d[:], stage[:])

    nc.compile()
    return nc


def _host_prep(risk, event_indicator, time, n_cores, K, HK):
    """Shard + dtype-convert inputs; returns per-core in_maps."""
    tnext = np.empty_like(time)
    tnext[:-1] = time[1:]
    tnext[-1] = time[-1] + 1
    t16 = time.astype(np.int16)
    tn16 = tnext.astype(np.int16)
    # fix any int16 aliasing so (t16==tn16) <=> (time==tnext)
    bad = (tnext != time) & (tn16 == t16)
    if bad.any():
        tn16[bad] = (t16[bad] + 1).astype(np.int16)
    ev16 = event_indicator.astype(ml_dtypes.bfloat16)
    rk16 = risk.astype(ml_dtypes.bfloat16)

    # halo validation: each core's edge-spanning group must end in the halo
    for c in range(1, n_cores):
        e = c * K
        gend = np.searchsorted(time, time[e], side="right") - 1
        if gend >= e + HK - 1:
            raise RuntimeError(
                f"halo too small: group at core edge {c} ends at {gend}"
            )

    m1 = np.triu(np.ones((P, P), np.float32), 1)  # m1[q, m] = 1 if q < m
    eye = np.eye(P, dtype=np.float32)
    ones1 = np.ones((1, P), np.float32)

    # sentinel halo content (every element a boundary, risk 0)
    sent_r = np.zeros(HK, ml_dtypes.bfloat16)
    sent_t = np.zeros(HK, np.int16)
    sent_n = np.ones(HK, np.int16)

    in_maps = []
    for c in range(n_cores):
        sl = slice(c * K, (c + 1) * K)
        hs = slice((c + 1) * K, (c + 1) * K + HK)
        masklt = (np.arange(n_cores) < c).astype(np.float32).reshape(1, -1)
        if c < n_cores - 1:
            hr, ht, hn = rk16[hs], t16[hs], tn16[hs]
        else:
            hr, ht, hn = sent_r, sent_t, sent_n
        in_maps.append({
            "risk": np.ascontiguousarray(rk16[sl]),
            "event": np.ascontiguousarray(ev16[sl]),
            "t16": np.ascontiguousarray(t16[sl]),
            "tn16": np.ascontiguousarray(tn16[sl]),
            "hrisk": np.ascontiguousarray(hr),
            "ht16": np.ascontiguousarray(ht),
            "htn16": np.ascontiguousarray(hn),
            "m1": m1, "eye": eye, "ones1": ones1,
            "masklt": masklt,
        })
    return in_maps


_NC_CACHE = {}


def _get_nc(n_cores, K, F):
    key = (n_cores, K, F)
    if key not in _NC_CACHE:
        _NC_CACHE[key] = build_nc(n_cores, K, F)
    return _NC_CACHE[key]


def run(risk, event_indicator, time, n_cores=NCORES_FULL, F=4096, **spmd_kwargs):
    from concourse.bass_utils import run_bass_kernel_spmd

    n = risk.shape[0]
    K = n // n_cores
    FT = K // P
    HK = P * (HW_HALO if FT >= 4 * F else 32)
    nc = _get_nc(n_cores, K, F)
    in_maps = _host_prep(risk, event_indicator, time, n_cores, K, HK)
    res = run_bass_kernel_spmd(
        nc, in_maps, core_ids=list(range(n_cores)), **spmd_kwargs
    )
    outs = np.stack([r["out"][0] for r in res.results])  # [n_cores, 64]
    A = outs[:, 0].astype(np.float64).sum()
    B = outs[:, 1].astype(np.float64).sum()
    loss = -(A - B) / n
    return np.float32(loss), res


def kernel(risk, event_indicator, time):
    loss, _ = run(risk, event_indicator, time)
    return np.asarray(loss, dtype=np.float32)


# revision 21
# speedup vs baseline: 1.4856x; 1.1699x over previous
"""Cox partial-likelihood NLL loss on 8 Trainium2 NeuronCores.

Math: with time sorted ascending and c = cumsum(exp(risk)),
    end(i)  = last index of i's tie group
    loss    = -(A - B) / N
    A       = sum_i event[i] * risk[i]
    B       = sum_i event[i] * ln(c[end(i)])

c[end(i)] = min over group-end positions k >= i of c[k] (c is increasing).
Device computes, per core (contiguous chunk, partition-major layout):
  s = exp(risk) (accum -> S_c, AllGathered early, overlapped with compute)
  cs = partition-local forward add-scan of s
  mb = cs + 1e30 * [time[i] == time[i+1]]     (finite only at group ends)
  bf = reverse min-scan of mb per tile, then hierarchical suffix-min fixup
       (tile suffix -> partition suffix; cross-core handled by a HALO tile:
        the next core's first H elements are re-processed locally, so the
        fill value for this core's tail is found without exchanging mins)
  B  = sum event * ln(bf + rowbase + corebase)   (STT product + accum)
  A  = sum event * risk                          (PE diag-block matmuls)
Host sums the 8 per-core (A_c, B_c) partials.
"""

import numpy as np
import ml_dtypes

N_FULL = 16_777_216
NCORES_FULL = 8
P = 128

BIG = 1.0e30      # mask offset for non-boundary positions
BIGF = 3.0e38     # "+inf" for f32 min chains
HW_HALO = 128     # halo tile free-width (halo = 128*HW_HALO elements)


def build_nc(n_cores: int, K: int, F: int):
    """Build the Bass module for per-core chunk length K, tile free-size F."""
    import concourse.bacc as bacc
    import concourse.tile as tile
    import concourse.mybir as mybir

    f32 = mybir.dt.float32
    bf16 = mybir.dt.bfloat16
    i16 = mybir.dt.int16
    Alu = mybir.AluOpType
    Act = mybir.ActivationFunctionType
    X = mybir.AxisListType.X

    FT = K // P          # elements per partition
    assert FT * P == K
    # ramp-up schedule: small leading tiles so compute starts early
    tiles = []
    off = 0
    ramp = [512, 512, 1024, 2048]
    for w in ramp:
        if off + w <= FT and FT >= 4 * F:
            tiles.append((off, w))
            off += w
    while off < FT:
        w = min(F, FT - off)
        tiles.append((off, w))
        off += w
    TM_ = len(tiles)         # number of MAIN tiles
    T = TM_ + 1              # + halo tile
    HW = HW_HALO if FT >= 4 * F else 32
    HK = P * HW              # halo element count

    nc = bacc.Bacc(
        "TRN2",
        target_bir_lowering=False,
        debug=False,
        enable_asserts=False,
        num_devices=n_cores,
    )

    risk_d = nc.dram_tensor("risk", [K], bf16, kind="ExternalInput").ap()
    event_d = nc.dram_tensor("event", [K], bf16, kind="ExternalInput").ap()
    t16_d = nc.dram_tensor("t16", [K], i16, kind="ExternalInput").ap()
    tn16_d = nc.dram_tensor("tn16", [K], i16, kind="ExternalInput").ap()
    hrisk_d = nc.dram_tensor("hrisk", [HK], bf16, kind="ExternalInput").ap()
    ht16_d = nc.dram_tensor("ht16", [HK], i16, kind="ExternalInput").ap()
    htn16_d = nc.dram_tensor("htn16", [HK], i16, kind="ExternalInput").ap()
    # constants / per-core masks
    m1_d = nc.dram_tensor("m1", [P, P], f32, kind="ExternalInput").ap()
    eye_d = nc.dram_tensor("eye", [P, P], f32, kind="ExternalInput").ap()
    ones1_d = nc.dram_tensor("ones1", [1, P], f32, kind="ExternalInput").ap()
    masklt_d = nc.dram_tensor("masklt", [n_cores, 1], f32, kind="ExternalInput").ap()
    out_d = nc.dram_tensor("out", [1, 64], f32, kind="ExternalOutput").ap()

    risk2 = risk_d.rearrange("(p f) -> p f", p=P)
    event2 = event_d.rearrange("(p f) -> p f", p=P)
    t162 = t16_d.rearrange("(p f) -> p f", p=P)
    tn162 = tn16_d.rearrange("(p f) -> p f", p=P)
    hrisk2 = hrisk_d.rearrange("(p f) -> p f", p=P)
    ht162 = ht16_d.rearrange("(p f) -> p f", p=P)
    htn162 = htn16_d.rearrange("(p f) -> p f", p=P)

    with tile.TileContext(nc) as tc:
        with (
            tc.tile_pool(name="pers", bufs=1) as pers,
            tc.tile_pool(name="io", bufs=2) as io,
            tc.tile_pool(name="sp", bufs=1) as sp,
            tc.tile_pool(name="pp", bufs=1, space="PSUM") as pp,
            tc.tile_pool(name="dram", bufs=1, space="DRAM") as dram,
        ):
            # ---- persistent SBUF ----
            bf0 = pers.tile([P, FT], bf16)         # mb -> bf (in place)
            event_sb = pers.tile([P, FT], bf16)
            TM = pers.tile([P, TM_], f32)          # per-tile row mins (main)
            RS = pers.tile([P, TM_], f32)          # suffix mins over tiles
            ciloc = pers.tile([P, TM_], f32)       # per-(partition,tile) init
            Bacc = pers.tile([P, TM_], f32)        # per-tile B partial sums
            Eacc = pers.tile([P, TM_], f32)        # per-tile exp row sums
            m1 = pers.tile([P, P], f32)
            eye = pers.tile([P, P], f32)
            ones1 = pers.tile([1, P], f32)
            masklt = pers.tile([n_cores, 1], f32)
            rowbase = pers.tile([P, 1], f32)       # excl prefix of partition totals
            bias128 = pers.tile([P, 1], f32)       # rowbase + base_c
            initloc = pers.tile([P, 1], f32)
            g128 = pers.tile([P, 1], f32)
            exT = pers.tile([1, P], f32)
            erow = pers.tile([P, 1], f32)          # per-partition exp sums
            hacc = pers.tile([P, 1], f32)          # halo per-row exp sums
            hrb = pers.tile([P, 1], f32)           # halo row bases
            hmb = pers.tile([P, HW], bf16)         # halo masked values
            hcs = pers.tile([P, HW], f32)
            hmin = pers.tile([P, 1], f32)
            S8T = pers.tile([n_cores, 1], f32)
            ejunk = pers.tile([P, TM_], f32)
            tjunk = pers.tile([1, P], f32)
            stage = pers.tile([1, 64], f32)        # collective-in / output staging
            scal = pers.tile([1, 8], f32)          # small scalar scratch (p0)
            tmpd = pers.tile([P, P], f32)
            dA = pers.tile([P, 1], f32)
            dB = pers.tile([P, 1], f32)

            # ---- PSUM ----
            psumA = pp.tile([P, P], f32)
            psumP = pp.tile([P, 1], f32)
            psumT = pp.tile([1, P], f32)
            psumI = pp.tile([P, 1], f32)
            psumS = pp.tile([1, 1], f32)

            # ---- DRAM bounce for the collective ----
            cc_in = dram.tile([1, 64], f32)
            cc_out = dram.tile([n_cores, 64], f32)

            nc.vector.memset(scal[:], 0.0)
            nc.vector.memset(Bacc[:], 0.0)
            nc.vector.memset(Eacc[:], 0.0)
            # load constants (small)
            nc.sync.dma_start(m1[:], m1_d[:])
            nc.sync.dma_start(eye[:], eye_d[:])
            nc.sync.dma_start(ones1[:], ones1_d[:])
            nc.sync.dma_start(masklt[:], masklt_d[:])

            # ================= phase 1: streaming =================
            cs_prev = None
            w_prev = None

            for t, (off, w) in enumerate(tiles):
                sl = slice(off, off + w)
                rbf_t = io.tile([P, w], bf16, tag="rbf")
                t16_t = io.tile([P, w], i16, tag="t16")
                tn16_t = io.tile([P, w], i16, tag="tn16")
                eq_t = io.tile([P, w], bf16, tag="eq")
                s_t = sp.tile([P, w], f32, tag="s")
                cs_t = io.tile([P, w], f32, tag="cs")

                nc.sync.dma_start(rbf_t[:], risk2[:, sl])
                nc.sync.dma_start(t16_t[:], t162[:, sl])
                nc.sync.dma_start(tn16_t[:], tn162[:, sl])
                nc.sync.dma_start(event_sb[:, sl], event2[:, sl])

                # s = exp(risk); row sums accumulate toward S_c
                nc.scalar.activation(
                    s_t[:], rbf_t[:], Act.Exp, accum_out=Eacc[:, t : t + 1]
                )
                # cs = forward add-scan of s (chained across tiles)
                init = 0.0 if cs_prev is None else cs_prev[:, w_prev - 1 : w_prev]
                nc.vector.tensor_tensor_scan(
                    cs_t[:], s_t[:], s_t[:], init, Alu.add, Alu.bypass
                )
                # eq = (t16 == tn16)  {1.0 interior, 0.0 at group end}
                nc.vector.tensor_tensor(eq_t[:], t16_t[:], tn16_t[:], Alu.is_equal)
                # mb = eq*BIG + cs   (bf16)
                nc.vector.scalar_tensor_tensor(
                    bf0[:, sl], eq_t[:], BIG, cs_t[:], Alu.mult, Alu.add
                )
                # bf0 = reverse min-scan of mb within the tile (in place)
                rev = bf0[:, sl][:, ::-1]
                nc.vector.tensor_tensor_scan(
                    rev, rev, rev, BIGF, Alu.min, Alu.bypass
                )
                # tile row-min = leftmost element of the reverse scan
                nc.vector.tensor_copy(TM[:, t : t + 1], bf0[:, off : off + 1])

                # A += event_blk . risk_blk (diagonal blocks, accumulate)
                for b in range(w // P):
                    bsl = slice(off + b * P, off + (b + 1) * P)
                    nc.tensor.matmul(
                        psumA[:],
                        event_sb[:, bsl],
                        rbf_t[:, b * P : (b + 1) * P],
                        start=(t == 0 and b == 0),
                        stop=(t == TM_ - 1 and b == w // P - 1),
                        skip_group_check=True,
                    )
                cs_prev = cs_t
                w_prev = w

            # ---- early collective: AllGather core sums S_c (overlapped) ----
            # Staging runs on ACT/PE so it does not queue behind phase-1 DVE.
            nc.scalar.activation(ejunk[:], Eacc[:], Act.Identity,
                                 accum_out=erow[:])
            nc.tensor.transpose(psumT[:], erow[:], eye[:])
            nc.scalar.activation(tjunk[:], psumT[:], Act.Identity,
                                 accum_out=scal[:, 0:1])
            nc.gpsimd.memset(stage[:], 0.0)
            nc.scalar.copy(stage[:, 0:1], scal[:, 0:1])
            nc.sync.dma_start(cc_in[:], stage[:])
            nc.gpsimd.collective_compute(
                "AllGather",
                Alu.bypass,
                replica_groups=[list(range(n_cores))],
                ins=[cc_in[:].opt()],
                outs=[cc_out[:].opt()],
            )
            # base_c = sum over cores < me of S, via PE: S8T.T @ maskltT
            nc.sync.dma_start(S8T[:], cc_out[:, 0:1])
            nc.tensor.matmul(psumS[:], S8T[:], masklt[:], start=True,
                             stop=True, skip_group_check=True)
            nc.scalar.copy(scal[:, 2:3], psumS[:])

            # ---- halo chunk (next core's first HK elements) ----
            # Scan it in the true core-global frame: row q's initial is
            # S_local + sum of halo rows < q. Its masked min M_halo is the
            # fill floor for this core's tail (replaces a cross-core min
            # exchange).
            hrbf = io.tile([P, HW], bf16, tag="rbf")
            ht16 = io.tile([P, HW], i16, tag="t16")
            htn16 = io.tile([P, HW], i16, tag="tn16")
            heq = io.tile([P, HW], bf16, tag="eq")
            nc.sync.dma_start(hrbf[:], hrisk2[:, :])
            nc.sync.dma_start(ht16[:], ht162[:, :])
            nc.sync.dma_start(htn16[:], htn162[:, :])
            nc.scalar.activation(hcs[:], hrbf[:], Act.Exp, accum_out=hacc[:])
            # halo row bases: strict-lower prefix of hacc + S_local broadcast
            nc.tensor.matmul(psumI[:], m1[:], hacc[:], start=True, stop=False,
                             skip_group_check=True)
            nc.tensor.matmul(psumI[:], ones1[:], scal[:, 0:1], start=False,
                             stop=True, skip_group_check=True)
            nc.scalar.copy(hrb[:], psumI[:])
            nc.vector.tensor_tensor_scan(
                hcs[:], hcs[:], hcs[:], hrb[:, 0:1], Alu.add, Alu.bypass
            )
            nc.vector.tensor_tensor(heq[:], ht16[:], htn16[:], Alu.is_equal)
            nc.vector.scalar_tensor_tensor(
                hmb[:], heq[:], BIG, hcs[:], Alu.mult, Alu.add
            )
            nc.vector.tensor_reduce(hmin[:], hmb[:], X, Alu.min)
            nc.tensor.transpose(psumT[:], hmin[:], eye[:])
            nc.vector.tensor_reduce(scal[:, 5:6], psumT[:], X, Alu.min)

            # ================= mid phase: local-only cross ops ==========
            # rowbase = excl prefix over partitions of MAIN row totals (erow;
            # ACT-accumulated, ~= scan totals to within fp rounding).
            nc.tensor.matmul(psumP[:], m1[:], erow[:], start=True, stop=True,
                             skip_group_check=True)
            nc.scalar.copy(rowbase[:], psumP[:])
            # suffix mins over tiles within each partition
            nc.vector.tensor_tensor_scan(
                RS[:, ::-1], TM[:, ::-1], TM[:, ::-1], BIGF, Alu.min, Alu.bypass
            )
            # whole-core row mins in core-local frame: g = RS[:,0] + rowbase
            nc.vector.tensor_tensor(g128[:], RS[:, 0:1], rowbase[:], Alu.add)
            nc.tensor.transpose(psumT[:], g128[:], eye[:])
            # partition-suffix mins, exclusive, floor M_halo:
            # exT[p] = min(min over q>p of gT[q], M_halo)
            nc.vector.tensor_tensor_scan(
                exT[:, 0 : P - 1][:, ::-1],
                psumT[:, 1:P][:, ::-1],
                eye[0:1, 0 : P - 1],
                scal[:, 5:6], Alu.min, Alu.bypass,
            )
            nc.vector.tensor_copy(exT[:, P - 1 : P], scal[:, 5:6])
            nc.tensor.transpose(psumI[:], exT[:], eye[0:1, 0:1])
            nc.vector.tensor_tensor(initloc[:], psumI[:], rowbase[:], Alu.subtract)
            # bias128 = rowbase + base_c (broadcast via PE ones)
            nc.tensor.matmul(psumP[:], ones1[:], scal[:, 2:3], start=True,
                             stop=True, skip_group_check=True)
            nc.vector.tensor_tensor(bias128[:], rowbase[:], psumP[:], Alu.add)
            # ciloc[:, t] = min(RS[:, t+1], initloc); last tile: initloc only
            nc.vector.memset(ciloc[:], BIGF)
            if TM_ > 1:
                nc.vector.tensor_copy(ciloc[:, 0 : TM_ - 1], RS[:, 1:TM_])
            nc.vector.tensor_scalar(
                ciloc[:], ciloc[:], initloc[:], None, Alu.min
            )

            # ================= phase 2: fix up + Ln + B accum ===========
            for t, (off, w) in enumerate(tiles):
                sl = slice(off, off + w)
                lbf_t = io.tile([P, w], bf16, tag="lbf")
                nc.vector.tensor_scalar(
                    bf0[:, sl], bf0[:, sl], ciloc[:, t : t + 1], None, Alu.min
                )
                nc.scalar.activation(
                    lbf_t[:], bf0[:, sl], Act.Ln, bias=bias128[:, 0:1], scale=1.0
                )
                nc.vector.scalar_tensor_tensor(
                    lbf_t[:], lbf_t[:], 0.0, event_sb[:, sl],
                    Alu.bypass, Alu.mult,
                    accum_out=Bacc[:, t : t + 1],
                )

            # ================= epilogue: reduce A and B =================
            nc.vector.tensor_tensor(tmpd[:], psumA[:], eye[:], Alu.mult)
            nc.vector.tensor_reduce(dA[:], tmpd[:], X, Alu.add)
            nc.vector.tensor_reduce(dB[:], Bacc[:], X, Alu.add)
            nc.vector.memset(stage[:], 0.0)
            nc.tensor.transpose(psumT[:], dA[:], eye[:])
            nc.vector.tensor_reduce(stage[:, 0:1], psumT[:], X, Alu.add)
            nc.tensor.transpose(psumT[:], dB[:], eye[:])
            nc.vector.tensor_reduce(stage[:, 1:2], psumT[:], X, Alu.add)
            nc.vector.tensor_copy(stage[:, 2:4], scal[:, 0:2])
            nc.vector.tensor_copy(stage[:, 4:5], scal[:, 2:3])
            nc.sync.dma_start(out_d[:], stage[:])

    nc.compile()
    return nc


def _host_prep(risk, event_indicator, time, n_cores, K, HK):
    """Shard + dtype-convert inputs; returns per-core in_maps."""
    tnext = np.empty_like(time)
    tnext[:-1] = time[1:]
    tnext[-1] = time[-1] + 1
    t16 = time.astype(np.int16)
    tn16 = tnext.astype(np.int16)
    # fix any int16 aliasing so (t16==tn16) <=> (time==tnext)
    bad = (tnext != time) & (tn16 == t16)
    if bad.any():
        tn16[bad] = (t16[bad] + 1).astype(np.int16)
    ev16 = event_indicator.astype(ml_dtypes.bfloat16)
    rk16 = risk.astype(ml_dtypes.bfloat16)

    # halo validation: each core's edge-spanning group must end in the halo
    for c in range(1, n_cores):
        e = c * K
        gend = np.searchsorted(time, time[e], side="right") - 1
        if gend >= e + HK - 1:
            raise RuntimeError(
                f"halo too small: group at core edge {c} ends at {gend}"
            )

    m1 = np.triu(np.ones((P, P), np.float32), 1)  # m1[q, m] = 1 if q < m
    eye = np.eye(P, dtype=np.float32)
    ones1 = np.ones((1, P), np.float32)

    # sentinel halo content (every element a boundary, risk 0)
    sent_r = np.zeros(HK, ml_dtypes.bfloat16)
    sent_t = np.zeros(HK, np.int16)
    sent_n = np.ones(HK, np.int16)

    in_maps = []
    for c in range(n_cores):
        sl = slice(c * K, (c + 1) * K)
        hs = slice((c + 1) * K, (c + 1) * K + HK)
        masklt = (np.arange(n_cores) < c).astype(np.float32).reshape(-1, 1)
        if c < n_cores - 1:
            hr, ht, hn = rk16[hs], t16[hs], tn16[hs]
        else:
            hr, ht, hn = sent_r, sent_t, sent_n
        in_maps.append({
            "risk": np.ascontiguousarray(rk16[sl]),
            "event": np.ascontiguousarray(ev16[sl]),
            "t16": np.ascontiguousarray(t16[sl]),
            "tn16": np.ascontiguousarray(tn16[sl]),
            "hrisk": np.ascontiguousarray(hr),
            "ht16": np.ascontiguousarray(ht),
            "htn16": np.ascontiguousarray(hn),
            "m1": m1, "eye": eye, "ones1": ones1,
            "masklt": masklt,
        })
    return in_maps


_NC_CACHE = {}


def _get_nc(n_cores, K, F):
    key = (n_cores, K, F)
    if key not in _NC_CACHE:
        _NC_CACHE[key] = build_nc(n_cores, K, F)
    return _NC_CACHE[key]


def run(risk, event_indicator, time, n_cores=NCORES_FULL, F=4096, **spmd_kwargs):
    from concourse.bass_utils import run_bass_kernel_spmd

    n = risk.shape[0]
    K = n // n_cores
    FT = K // P
    HK = P * (HW_HALO if FT >= 4 * F else 32)
    nc = _get_nc(n_cores, K, F)
    in_maps = _host_prep(risk, event_indicator, time, n_cores, K, HK)
    res = run_bass_kernel_spmd(
        nc, in_maps, core_ids=list(range(n_cores)), **spmd_kwargs
    )
    outs = np.stack([r["out"][0] for r in res.results])  # [n_cores, 64]
    A = outs[:, 0].astype(np.float64).sum()
    B = outs[:, 1].astype(np.float64).sum()
    loss = -(A - B) / n
    return np.float32(loss), res


def kernel(risk, event_indicator, time):
    loss, _ = run(risk, event_indicator, time)
    return np.asarray(loss, dtype=np.float32)
